# revision 7
# baseline (speedup 1.0000x reference)
"""Bidirectional simplified SSM kernel for Trainium2 (8 NeuronCores).

Math (per batch element b):
    z = x @ W_in                                  [L, DI]
    fwd:  o = z @ W_fwd; delta = sigmoid(o[:, :DI]); gate = o[:, DI:] * z
          h_t = delta_t * h_{t-1} + gate_t        (t ascending)
    bwd:  same with W_bwd, t descending
    y    = concat(h_fwd, h_bwd) @ W_out + x
    out  = LayerNorm(y) * gamma + beta

Sharding: 8 cores = 4 batches x 2 sequence halves with a 32-token halo on
each side (sigmoid gating decays ~0.5/step, so 32 warm-up steps reproduce
the cross-half scan state far below fp16 noise). No cross-core traffic.

v3 strategy (v1 = 99.4us):
- Zero-mean trick: host centers each W_out row over its 2048 outputs and
  folds per-token x-centering into the residual lo plane, so y has
  analytically zero mean and LayerNorm reduces to out = py * rstd (no mean
  pass, no center pass).
- All GEMMs fp8 DoubleRow (xT8 = fp8(32x) D-major; w_in8 + w_in8l e5m2
  residual = 16 W_in; Wf8/Wb8 = 64 W; scan state fp32 on DVE, h8 = fp8(8h);
  W_out8 = fp8(16 W'_out)). Residual adds are DoubleRow matmuls through
  stacked-identity selectors [4I;0]/[0;4I]: hi plane extracted from
  resident xT8 kblock pairs, lo plane from pair-packed r8p.
- Phase A software-pipelined: z GEMM of slab i+1 runs between z(i) and
  o(i) so the PE never waits for the z8 psum->fp8 copy (DVE).
- Phase B (bwd) runs right before phase C (interleaving its sigmoids with
  phase C's Sqrt ops would thrash the ACT table sets); its o-GEMM/sigmoid/
  gate work drains while phase C's first chunk GEMMs run.
- Phase C per chunk, stats arranged so only a 512-col Square is on the
  psum-release critical chain: DVE tensor_tensor_reduce squares psum half
  0 (under half 1's GEMMs) and bank (1,0) (under bank (1,1)'s GEMMs,
  chaining partial sums via the reduce init); ACT squares the last bank;
  ACT Sqrt folds the off-chain partial via bias = 0.5*ss01 + eps; DVE
  recip -> rstd; normalize py*rstd: ACT (half 0, Copy with scale=rstd)
  and Pool (half 1); y stored fp16.
- Work split: gates (scalar_tensor_tensor) on Pool, z8 copies on DVE,
  identities built on DVE (no ACT table churn before the sigmoid set).
"""

import os
import sys

for _p in ("/opt/trn_rl_repo", "/root/.axon_site/_ro/trn_rl_repo"):
    if os.path.isdir(_p) and _p not in sys.path:
        sys.path.insert(0, _p)

import ml_dtypes
import numpy as np

import concourse.bacc as bacc
import concourse.mybir as mybir
import concourse.tile as tile
from concourse.masks import make_identity

P = 128
LN_EPS = 1e-5

B, L, D, DI = 4, 4096, 2048, 256
HALO = 32
T_OWN = L // 2            # tokens owned per core (2048)
T_CTX = T_OWN + 2 * HALO  # context incl. halos (2112)
T_SCAN = T_CTX - HALO     # tokens each direction scans over (2080)
N_CORES = 8

KD = D // P               # 16 k-blocks over D
NCH = T_OWN // P          # 16 output chunks
NCH2 = NCH // 2           # 8 chunk pairs for the lo-plane DR packing
SC = 128.0                # psum y scale (8*16)
EPS_S = SC * SC * LN_EPS

F8 = ml_dtypes.float8_e4m3

f8 = mybir.dt.float8e4
f8e5 = mybir.dt.float8e5
f16 = mybir.dt.float16
f32 = mybir.dt.float32
AO = mybir.AluOpType
AF = mybir.ActivationFunctionType
DR = mybir.MatmulPerfMode.DoubleRow

# segment tables (token coordinates); T_CTX=2112, T_SCAN=2080 with HALO=32
SLABS = [(0, 512), (512, 512), (1024, 512), (1536, 512), (2048, 64)]
SLAB_DMA = {0: (0, 512), 1: (512, 512), 2: (1024, 512), 3: (1536, 576)}
FSEGS = [(0, 512), (512, 512), (1024, 512), (1536, 512), (2048, 32)]
BSEGS = [(2048, 32), (1536, 512), (1024, 512), (512, 512), (0, 512)]


def build_nc():
    nc = bacc.Bacc("TRN2", target_bir_lowering=False, debug=False)
    xT8_d = nc.dram_tensor("xT8", [P, KD, T_CTX], f8, kind="ExternalInput").ap()
    r8p_d = nc.dram_tensor("r8p", [P, NCH2, 2, D], f8,
                           kind="ExternalInput").ap()
    win_d = nc.dram_tensor("w_in8", [P, 2, KD, DI // 2], f8,
                           kind="ExternalInput").ap()
    winl_d = nc.dram_tensor("w_in8l", [P, 2, KD, DI // 2], f8e5,
                            kind="ExternalInput").ap()
    wf_d = nc.dram_tensor("w_f8", [P, 2, 2 * DI], f8, kind="ExternalInput").ap()
    wb_d = nc.dram_tensor("w_b8", [P, 2, 2 * DI], f8, kind="ExternalInput").ap()
    wo_d = nc.dram_tensor("w_o8", [P, 4, D], f8, kind="ExternalInput").ap()
    y_d = nc.dram_tensor("y", [T_OWN, D], f16, kind="ExternalOutput").ap()

    with tile.TileContext(nc) as tc:
        with (
            tc.tile_pool(name="const", bufs=1) as cpool,
            tc.tile_pool(name="big", bufs=1) as bpool,
            tc.tile_pool(name="ych", bufs=3) as ypool,
            tc.tile_pool(name="st", bufs=4) as stpool,
            tc.tile_pool(name="ps", bufs=4, space="PSUM") as psp,
        ):
            # ---- constants / weights ----
            w_in8 = cpool.tile([P, 2, KD, DI // 2], f8)
            w_in8l = cpool.tile([P, 2, KD, DI // 2], f8e5)
            w_f8 = cpool.tile([P, 2, 2 * DI], f8)
            w_b8 = cpool.tile([P, 2, 2 * DI], f8)
            w_o8 = cpool.tile([P, 4, D], f8)
            ident = cpool.tile([P, P], f16)
            identhi8 = cpool.tile([P, 2, P], f8)   # [4I; 0]
            identlo8 = cpool.tile([P, 2, P], f8)   # [0; 4I]
            eps_t = cpool.tile([P, 1], f32)
            nc.sync.dma_start(w_in8[:, 0], win_d[:, 0])
            nc.sync.dma_start(w_in8l[:, 0], winl_d[:, 0])
            nc.sync.dma_start(w_in8[:, 1], win_d[:, 1])
            nc.sync.dma_start(w_in8l[:, 1], winl_d[:, 1])
            make_identity(nc, ident[:])
            nc.vector.memset(identhi8[:], 0.0)
            nc.vector.memset(identlo8[:], 0.0)
            nc.vector.tensor_scalar(identhi8[:, 0], ident[:], 4.0, None,
                                    AO.mult)
            nc.vector.tensor_scalar(identlo8[:, 1], ident[:], 4.0, None,
                                    AO.mult)
            nc.vector.memset(eps_t[:], EPS_S)

            # ---- big SBUF state ----
            xT8 = bpool.tile([P, KD, T_CTX], f8)
            r8p = bpool.tile([P, NCH2, 2, D], f8)
            z8 = bpool.tile([P, 2, T_CTX], f8)
            d16 = bpool.tile([P, 2, T_SCAN], f16)   # fwd delta
            g16 = bpool.tile([P, 2, T_SCAN], f16)   # fwd gate
            d16b = bpool.tile([P, 2, T_SCAN], f16)  # bwd delta
            g16b = bpool.tile([P, 2, T_SCAN], f16)  # bwd gate
            h8f = bpool.tile([P, 2, T_SCAN], f8)
            h8b = bpool.tile([P, 2, T_SCAN], f8)
            sqscr = bpool.tile([P, 2, 512], f16)    # ACT square scratch
            sqscrd = bpool.tile([P, 2, 512], f16)   # DVE square scratch

            # ---- phase A: z GEMM + fwd direction, software-pipelined ----
            def z_slab(si):
                t0, ts = SLABS[si]
                pz = psp.tile([P, 2, 512], f32, name="pz", tag="ps")
                for m in range(2):
                    for j in range(KD // 2):
                        nc.tensor.matmul(
                            pz[:, m, :ts],
                            w_in8[:, m, 2 * j:2 * j + 2, :],
                            xT8[:, 2 * j:2 * j + 2, t0:t0 + ts],
                            start=(j == 0), stop=False,
                            perf_mode=DR,
                        )
                    for j in range(KD // 2):
                        nc.tensor.matmul(
                            pz[:, m, :ts],
                            w_in8l[:, m, 2 * j:2 * j + 2, :],
                            xT8[:, 2 * j:2 * j + 2, t0:t0 + ts],
                            start=False, stop=(j == KD // 2 - 1),
                            perf_mode=DR, skip_group_check=True,
                        )
                nc.vector.tensor_scalar(z8[:, :, t0:t0 + ts], pz[:, :, :ts],
                                        1.0 / 32.0, None, AO.mult)

            def fwd_seg(si):
                f0, fs = FSEGS[si]
                pod = psp.tile([P, 2, 512], f32, name="pod", tag="ps")
                pog = psp.tile([P, 2, 512], f32, name="pog", tag="ps")
                for m2 in range(4):
                    dst = pod if m2 < 2 else pog
                    nc.tensor.matmul(
                        dst[:, m2 % 2, :fs],
                        w_f8[:, :, m2 * P:(m2 + 1) * P],
                        z8[:, :, f0:f0 + fs],
                        start=True, stop=True, perf_mode=DR,
                    )
                nc.scalar.activation(
                    d16[:, :, f0:f0 + fs], pod[:, :, :fs], AF.Sigmoid,
                    scale=1.0 / 1024.0,
                )
                nc.gpsimd.scalar_tensor_tensor(
                    g16[:, :, f0:f0 + fs], pog[:, :, :fs],
                    1.0 / 2048.0, z8[:, :, f0:f0 + fs],
                    AO.mult, AO.mult,
                )
                for kb in range(2):
                    init = 0.0 if f0 == 0 else h8f[:, kb, f0 - 1:f0]
                    nc.vector.tensor_tensor_scan(
                        h8f[:, kb, f0:f0 + fs],
                        d16[:, kb, f0:f0 + fs],
                        g16[:, kb, f0:f0 + fs],
                        init, AO.mult, AO.add,
                    )

            nc.sync.dma_start(xT8[:, :, 0:512], xT8_d[:, :, 0:512])
            nc.sync.dma_start(w_f8[:], wf_d)
            for si in range(len(SLABS)):
                if si + 1 in SLAB_DMA:
                    t0, ts = SLAB_DMA[si + 1]
                    nc.sync.dma_start(xT8[:, :, t0:t0 + ts],
                                      xT8_d[:, :, t0:t0 + ts])
                z_slab(si)
                if si > 0:
                    fwd_seg(si - 1)
            fwd_seg(len(SLABS) - 1)

            # weights / lo-plane pairs for phases B/C (descending chunk order)
            nc.sync.dma_start(w_b8[:], wb_d)
            nc.sync.dma_start(w_o8[:], wo_d)
            for pc in (NCH2 - 1, NCH2 - 2):
                nc.sync.dma_start(r8p[:, pc], r8p_d[:, pc])

            # ---- phase B segs (emitted interleaved with phase C below) ----
            def bwd_seg(sj):
                b0, bs = BSEGS[sj]
                z0 = b0 + HALO
                pod = psp.tile([P, 2, 512], f32, name="pod", tag="ps")
                pog = psp.tile([P, 2, 512], f32, name="pog", tag="ps")
                for m2 in range(4):
                    dst = pod if m2 < 2 else pog
                    nc.tensor.matmul(
                        dst[:, m2 % 2, :bs],
                        w_b8[:, :, m2 * P:(m2 + 1) * P],
                        z8[:, :, z0:z0 + bs],
                        start=True, stop=True, perf_mode=DR,
                    )
                nc.scalar.activation(
                    d16b[:, :, b0:b0 + bs], pod[:, :, :bs], AF.Sigmoid,
                    scale=1.0 / 1024.0,
                )
                nc.gpsimd.scalar_tensor_tensor(
                    g16b[:, :, b0:b0 + bs], pog[:, :, :bs],
                    1.0 / 2048.0, z8[:, :, z0:z0 + bs],
                    AO.mult, AO.mult,
                )
                for kb in range(2):
                    hi = b0 + bs
                    init = 0.0 if hi == T_SCAN else h8b[:, kb, hi:hi + 1]
                    nc.vector.tensor_tensor_scan(
                        h8b[:, kb, b0:b0 + bs][:, ::-1],
                        d16b[:, kb, b0:b0 + bs][:, ::-1],
                        g16b[:, kb, b0:b0 + bs][:, ::-1],
                        init, AO.mult, AO.add,
                    )

            for sj in range(len(BSEGS)):
                bwd_seg(sj)

            # ---- phase C: out chunks, software-pipelined ----
            state = {}   # c -> (halves, st, y16)
            order = list(range(NCH - 1, -1, -1))

            def gemm_bank(py, c, o, g, tok, csel, pc):
                dgi = o * 2 + g
                dsl = slice(dgi * 512, (dgi + 1) * 512)
                nc.tensor.matmul(py[:, g, :], hfv[c], w_o8[:, 0:2, dsl],
                                 start=True, stop=False, perf_mode=DR)
                nc.tensor.matmul(py[:, g, :], hbv[c], w_o8[:, 2:4, dsl],
                                 start=False, stop=False, perf_mode=DR)
                # residual hi: DR transpose-extract of xT8 kblock pairs
                for j in range(4):
                    pb = dgi * 4 + (j // 2) * 2
                    sel = identhi8 if j % 2 == 0 else identlo8
                    nc.tensor.matmul(
                        py[:, g, j * P:(j + 1) * P],
                        xT8[:, pb:pb + 2, tok],
                        sel[:],
                        start=False, stop=False,
                        perf_mode=DR, skip_group_check=True)
                # residual lo: DR select of the chunk pair plane
                nc.tensor.matmul(py[:, g, :], csel[:],
                                 r8p[:, pc, :, dsl],
                                 start=False, stop=True,
                                 perf_mode=DR, skip_group_check=True)

            hfv = {c: h8f[:, :, HALO + c * P:HALO + (c + 1) * P]
                   for c in range(NCH)}
            hbv = {c: h8b[:, :, c * P:(c + 1) * P] for c in range(NCH)}

            def stage0(c, prev):
                tok = slice(HALO + c * P, HALO + (c + 1) * P)
                csel = identhi8 if c % 2 == 0 else identlo8
                pc = c // 2
                st = stpool.tile([P, 6], f32, name="st")
                y16 = ypool.tile([P, 4, 512], f16, name="y16")
                py0 = psp.tile([P, 2, 512], f32, name="py0", tag="ps")
                gemm_bank(py0, c, 0, 0, tok, csel, pc)
                gemm_bank(py0, c, 0, 1, tok, csel, pc)
                py1 = psp.tile([P, 2, 512], f32, name="py1", tag="ps")
                gemm_bank(py1, c, 1, 0, tok, csel, pc)
                # ss0 = sum(py0^2)/1024 on DVE, overlapping bank (1,*) GEMMs
                nc.vector.tensor_tensor_reduce(
                    sqscrd[:], py0[:], py0[:], 1.0 / 1024.0,
                    0.0, AO.mult, AO.add, accum_out=st[:, 0:1],
                )
                gemm_bank(py1, c, 1, 1, tok, csel, pc)
                # ss01 = ss0 + sum(py1[:,0]^2)/1024, overlapping bank (1,1)
                nc.vector.tensor_tensor_reduce(
                    sqscrd[:, 0], py1[:, 0], py1[:, 0], 1.0 / 1024.0,
                    st[:, 0:1], AO.mult, AO.add, accum_out=st[:, 1:2],
                )
                if prev is not None:
                    recip(prev)
                # bias for Sqrt: 0.5*ss01 + eps (off the critical chain)
                nc.vector.tensor_scalar(st[:, 2:3], st[:, 1:2], 0.5, EPS_S,
                                        AO.mult, op1=AO.add)
                state[c] = ((py0, py1), st, y16)

            def stage2a(c):
                # last bank's squares + sqrt: ACT-only, deps one iteration old
                (py0, py1), st, y16 = state[c]
                nc.scalar.activation(
                    sqscr[:, 0], py1[:, 1], AF.Square, scale=1.0 / 32.0,
                    accum_out=st[:, 3:4],
                )
                nc.scalar.activation(st[:, 4:5], st[:, 3:4], AF.Sqrt,
                                     scale=0.5, bias=st[:, 2:3])

            def recip(c):
                st = state[c][1]
                nc.vector.reciprocal(st[:, 5:6], st[:, 4:5])

            def stage2b(c):
                (py0, py1), st, y16 = state.pop(c)
                nc.scalar.activation(
                    y16[:, 0:2, :], py0[:], AF.Copy, scale=st[:, 5:6])
                nc.gpsimd.tensor_scalar(
                    y16[:, 2:4, :], py1[:], st[:, 5:6], None, AO.mult)
                nc.sync.dma_start(y_d[c * P:(c + 1) * P, :], y16[:])

            for i in range(NCH + 2):
                if 1 <= i < NCH + 1:
                    stage2a(order[i - 1])
                if 2 <= i < NCH + 2:
                    stage2b(order[i - 2])
                if i < NCH:
                    c = order[i]
                    if c % 2 == 0 and c // 2 - 2 >= 0:
                        nc.sync.dma_start(r8p[:, c // 2 - 2],
                                          r8p_d[:, c // 2 - 2])
                    stage0(c, order[i - 1] if i >= 1 else None)
                elif i - 1 < NCH:
                    recip(order[i - 1])

    nc.compile()
    return nc


_NC_CACHE = {}


def _get_nc():
    if "nc" not in _NC_CACHE:
        _NC_CACHE["nc"] = build_nc()
    return _NC_CACHE["nc"]


def _prep_weights(W_in, W_fwd, W_bwd, W_out):
    W_in = np.asarray(W_in, np.float32)
    W_fwd = np.asarray(W_fwd, np.float32)
    W_bwd = np.asarray(W_bwd, np.float32)
    W_out = np.asarray(W_out, np.float32)
    # [P, 2, KD, 128]: w[p, m, j, i] = 16*W_in[j*128+p, m*128+i]
    w_in16s = (16.0 * W_in).reshape(KD, P, 2, DI // 2).transpose(1, 2, 0, 3)
    w_in16s = np.ascontiguousarray(w_in16s)
    w_in8 = w_in16s.astype(F8)
    w_in8l = (w_in16s - w_in8.astype(np.float32)).astype(
        ml_dtypes.float8_e5m2)
    w_f8 = (64.0 * W_fwd).reshape(2, P, 2 * DI).transpose(1, 0, 2)
    w_f8 = np.ascontiguousarray(w_f8).astype(F8)
    w_b8 = (64.0 * W_bwd).reshape(2, P, 2 * DI).transpose(1, 0, 2)
    w_b8 = np.ascontiguousarray(w_b8).astype(F8)
    # center each W_out row over its 2048 outputs -> out-term of y has
    # (analytically) zero mean; LN then needs no mean subtraction
    W_oc = W_out - W_out.mean(-1, keepdims=True)
    w_o8 = (16.0 * W_oc).reshape(4, P, D).transpose(1, 0, 2)
    w_o8 = np.ascontiguousarray(w_o8).astype(F8)
    return {
        "w_in8": w_in8, "w_in8l": np.ascontiguousarray(w_in8l),
        "w_f8": w_f8, "w_b8": w_b8, "w_o8": w_o8,
    }


def shard_inputs(x, W_in, W_fwd, W_bwd, W_out):
    xf = np.asarray(x, np.float32)
    xp = np.zeros((B, L + 2 * HALO, D), np.float32)
    xp[:, HALO:HALO + L] = xf
    wmaps = _prep_weights(W_in, W_fwd, W_bwd, W_out)
    in_maps = []
    for b in range(B):
        for h in range(2):
            ctx = xp[b, h * T_OWN:h * T_OWN + T_CTX]          # [T_CTX, D]
            xT8 = (32.0 * ctx.T).reshape(KD, P, T_CTX).transpose(1, 0, 2)
            xT8 = np.ascontiguousarray(xT8).astype(F8)        # [P, KD, T_CTX]
            own = xf[b, h * T_OWN:(h + 1) * T_OWN]            # [T_OWN, D]
            # residual: hi = x8h (from xT8); lo absorbs the per-token
            # centering: r = 32*(x - mean_d x) - x8h
            x8h = xT8.astype(np.float32).transpose(1, 0, 2).reshape(D, T_CTX)
            own8h = x8h[:, HALO:HALO + T_OWN].T               # [T_OWN, D] (32x)
            ownc = own - own.mean(-1, keepdims=True)
            r = 32.0 * ownc - own8h
            # pair-packed [P, NCH2, 2, D]: chunk 2pc+i, token p
            r8p = r.reshape(NCH2, 2, P, D).transpose(2, 0, 1, 3)
            r8p = np.ascontiguousarray(r8p).astype(F8)
            in_maps.append({"xT8": xT8, "r8p": r8p, **wmaps})
    return in_maps


def gather_outputs(results):
    out = np.empty((B, L, D), np.float32)
    for b in range(B):
        for h in range(2):
            out[b, h * T_OWN:(h + 1) * T_OWN] = (
                results[b * 2 + h]["y"].astype(np.float32)
            )
    return out


def run_on_hw(x, W_in, W_fwd, W_bwd, W_out, trace=False):
    from concourse.bass_utils import run_bass_kernel_spmd

    nc = _get_nc()
    in_maps = shard_inputs(x, W_in, W_fwd, W_bwd, W_out)
    res = run_bass_kernel_spmd(
        nc, in_maps, core_ids=list(range(N_CORES)), trace=trace
    )
    return gather_outputs(res.results), res


def kernel(x, W_in, W_fwd, W_bwd, W_out, gamma, beta):
    y, _ = run_on_hw(x, W_in, W_fwd, W_bwd, W_out)
    gamma = np.asarray(gamma, np.float32)
    beta = np.asarray(beta, np.float32)
    if not (np.all(gamma == 1.0) and np.all(beta == 0.0)):
        y = y * gamma + beta
    return y.astype(np.float32)


# revision 11
# speedup vs baseline: 1.1369x; 1.1369x over previous
"""Bidirectional simplified SSM kernel for Trainium2 (8 NeuronCores).

Math (per batch element b):
    z = x @ W_in                                  [L, DI]
    fwd:  o = z @ W_fwd; delta = sigmoid(o[:, :DI]); gate = o[:, DI:] * z
          h_t = delta_t * h_{t-1} + gate_t        (t ascending)
    bwd:  same with W_bwd, t descending
    y    = concat(h_fwd, h_bwd) @ W_out + x
    out  = LayerNorm(y) * gamma + beta

Sharding: 8 cores = 4 batches x 2 sequence halves with a 32-token halo on
each side (sigmoid gating decays ~0.5/step, so 32 warm-up steps reproduce
the cross-half scan state far below fp16 noise). No cross-core traffic.

v6 strategy (v1 = 99.4us):
- Zero-mean trick: host centers each W_out row over its 2048 outputs and
  folds per-token x-centering into the residual lo plane, so y has
  analytically zero mean and LayerNorm reduces to out = py * rstd (no
  mean pass, no center pass).
- All GEMMs fp8 DoubleRow (xT8 = fp8(32x) D-major; w_in8 + w_in8l e5m2
  residual = 16 W_in; Wf8/Wb8 = 64 W; scan state fp32 on DVE, h8 =
  fp8(8h); W_out8 = fp8(16 W'_out)). Residual adds are DoubleRow matmuls
  through stacked-identity selectors [4I;0]/[0;4I]: hi plane extracted
  from resident xT8 kblock pairs, lo plane from pair-packed r8p.
- Every PSUM tile is ONE bank (2KB) under a single 8-slot rotation: the
  Tile scheduler then never couples a slow consumer (Pool gate, norm) to
  an unrelated producer through slot reuse, and frees happen at the
  finest granularity.
- Phase A: 256-token slabs. Each slab does z GEMM + z8, the fwd
  o/sigmoid/gate/scan segment AND the bwd o/sigmoid/gate for the aligned
  bwd segment (scan-order-free), so all sigmoids finish before phase C's
  Sqrt cadence (one ACT table switch) and the bwd chain costs no
  dedicated phase. Only the serial bwd scans remain, overlapping phase
  C's first GEMMs.
- Phase C per chunk = 4 independent 512-col psum quarters. Squares:
  Q0/Q2 on DVE (tensor_tensor_reduce, init-chained), Q1 on Pool
  (scalar_tensor_tensor + accum), Q3 on ACT right when its GEMMs land;
  ACT Sqrt folds the other partials via bias = 0.5*(ss02+ss1) + eps
  (DVE smalls, off-chain); DVE recip; norms split ACT(Q0,Q2)/DVE(Q1)/
  Pool(Q3). Stats lag one pipeline stage, norms two, so each in-order
  engine queue only sees already-satisfied deps.
"""

import os
import sys

for _p in ("/opt/trn_rl_repo", "/root/.axon_site/_ro/trn_rl_repo"):
    if os.path.isdir(_p) and _p not in sys.path:
        sys.path.insert(0, _p)

import ml_dtypes
import numpy as np

import concourse.bacc as bacc
import concourse.mybir as mybir
import concourse.tile as tile
from concourse.masks import make_identity

P = 128
LN_EPS = 1e-5

B, L, D, DI = 4, 4096, 2048, 256
HALO = 32
T_OWN = L // 2            # tokens owned per core (2048)
T_CTX = T_OWN + 2 * HALO  # context incl. halos (2112)
T_SCAN = T_CTX - HALO     # tokens each direction scans over (2080)
N_CORES = 8

KD = D // P               # 16 k-blocks over D
NCH = T_OWN // P          # 16 output chunks
NCH2 = NCH // 2           # 8 chunk pairs for the lo-plane DR packing
SC = 128.0                # psum y scale (8*16)
EPS_S = SC * SC * LN_EPS

F8 = ml_dtypes.float8_e4m3

f8 = mybir.dt.float8e4
f8e5 = mybir.dt.float8e5
f16 = mybir.dt.float16
f32 = mybir.dt.float32
AO = mybir.AluOpType
AF = mybir.ActivationFunctionType
DR = mybir.MatmulPerfMode.DoubleRow

# 256-token z slabs (+64 halo tail); DMA in 512/576-token copies
SLABS = [(256 * k, 256) for k in range(8)] + [(2048, 64)]
SLAB_DMA = {0: (0, 512), 2: (512, 512), 4: (1024, 512), 6: (1536, 576)}
# fwd scan segs aligned to slabs (scan coords == z coords, 0..2080)
FSEGS = [(256 * k, 256) for k in range(8)] + [(2048, 32)]
# bwd segs in scan coords; z range = (b0+32, len), exactly slab si
BSEGS = [(0, 224)] + [(224 + 256 * m, 256) for m in range(7)] + [(2016, 64)]


def build_nc():
    nc = bacc.Bacc("TRN2", target_bir_lowering=False, debug=False)
    xT8_d = nc.dram_tensor("xT8", [P, KD, T_CTX], f8, kind="ExternalInput").ap()
    r8p_d = nc.dram_tensor("r8p", [P, NCH2, 2, D], f8,
                           kind="ExternalInput").ap()
    win_d = nc.dram_tensor("w_in8", [P, 2, KD, DI // 2], f8,
                           kind="ExternalInput").ap()
    winl_d = nc.dram_tensor("w_in8l", [P, 2, KD, DI // 2], f8e5,
                            kind="ExternalInput").ap()
    wf_d = nc.dram_tensor("w_f8", [P, 2, 2 * DI], f8, kind="ExternalInput").ap()
    wb_d = nc.dram_tensor("w_b8", [P, 2, 2 * DI], f8, kind="ExternalInput").ap()
    wo_d = nc.dram_tensor("w_o8", [P, 4, D], f8, kind="ExternalInput").ap()
    y_d = nc.dram_tensor("y", [T_OWN, D], f16, kind="ExternalOutput").ap()

    with tile.TileContext(nc) as tc:
        with (
            tc.tile_pool(name="const", bufs=1) as cpool,
            tc.tile_pool(name="big", bufs=1) as bpool,
            tc.tile_pool(name="ych", bufs=3) as ypool,
            tc.tile_pool(name="st", bufs=4) as stpool,
            tc.tile_pool(name="ps", bufs=8, space="PSUM") as psp,
        ):
            # ---- constants / weights ----
            w_in8 = cpool.tile([P, 2, KD, DI // 2], f8)
            w_in8l = cpool.tile([P, 2, KD, DI // 2], f8e5)
            w_f8 = cpool.tile([P, 2, 2 * DI], f8)
            w_b8 = cpool.tile([P, 2, 2 * DI], f8)
            w_o8 = cpool.tile([P, 4, D], f8)
            ident = cpool.tile([P, P], f16)
            identhi8 = cpool.tile([P, 2, P], f8)   # [4I; 0]
            identlo8 = cpool.tile([P, 2, P], f8)   # [0; 4I]
            nc.sync.dma_start(w_in8[:, 0], win_d[:, 0])
            make_identity(nc, ident[:])
            nc.vector.memset(identhi8[:], 0.0)
            nc.vector.memset(identlo8[:], 0.0)
            nc.vector.tensor_scalar(identhi8[:, 0], ident[:], 4.0, None,
                                    AO.mult)
            nc.vector.tensor_scalar(identlo8[:, 1], ident[:], 4.0, None,
                                    AO.mult)

            # ---- big SBUF state ----
            xT8 = bpool.tile([P, KD, T_CTX], f8)
            r8p = bpool.tile([P, NCH2, 2, D], f8)
            z8 = bpool.tile([P, 2, T_CTX], f8)
            d16 = bpool.tile([P, 2, T_SCAN], f16)   # fwd delta
            g16 = bpool.tile([P, 2, T_SCAN], f16)   # fwd gate
            d16b = bpool.tile([P, 2, T_SCAN], f16)  # bwd delta
            g16b = bpool.tile([P, 2, T_SCAN], f16)  # bwd gate
            h8f = bpool.tile([P, 2, T_SCAN], f8)
            h8b = bpool.tile([P, 2, T_SCAN], f8)
            sqscr = bpool.tile([P, 512], f16)       # ACT square scratch
            sqscrd = bpool.tile([P, 512], f16)      # DVE square scratch
            sqscrp = bpool.tile([P, 512], f16)      # Pool square scratch

            # ---- phase A: z GEMM + fwd segs + bwd part1, slab by slab ----
            def z_slab(si):
                t0, ts = SLABS[si]
                pz = psp.tile([P, 2, 256], f32, name="pz", tag="ps")
                for m in range(2):
                    for j in range(KD // 2):
                        nc.tensor.matmul(
                            pz[:, m, :ts],
                            w_in8[:, m, 2 * j:2 * j + 2, :],
                            xT8[:, 2 * j:2 * j + 2, t0:t0 + ts],
                            start=(j == 0), stop=False,
                            perf_mode=DR,
                        )
                    for j in range(KD // 2):
                        nc.tensor.matmul(
                            pz[:, m, :ts],
                            w_in8l[:, m, 2 * j:2 * j + 2, :],
                            xT8[:, 2 * j:2 * j + 2, t0:t0 + ts],
                            start=False, stop=(j == KD // 2 - 1),
                            perf_mode=DR, skip_group_check=True,
                        )
                nc.vector.tensor_scalar(z8[:, :, t0:t0 + ts], pz[:, :, :ts],
                                        1.0 / 32.0, None, AO.mult)

            def dir_seg(z0, zs, s0, wdir, dd, gg, scan):
                """o GEMM + sigmoid + gate for z8[z0:z0+zs] -> coords s0.."""
                pod = psp.tile([P, 2, 256], f32, name="pod", tag="ps")
                pog = psp.tile([P, 2, 256], f32, name="pog", tag="ps")
                for m2 in range(4):
                    dst = pod if m2 < 2 else pog
                    nc.tensor.matmul(
                        dst[:, m2 % 2, :zs],
                        wdir[:, :, m2 * P:(m2 + 1) * P],
                        z8[:, :, z0:z0 + zs],
                        start=True, stop=True, perf_mode=DR,
                    )
                nc.scalar.activation(
                    dd[:, :, s0:s0 + zs], pod[:, :, :zs], AF.Sigmoid,
                    scale=1.0 / 1024.0,
                )
                nc.gpsimd.scalar_tensor_tensor(
                    gg[:, :, s0:s0 + zs], pog[:, :, :zs],
                    1.0 / 2048.0, z8[:, :, z0:z0 + zs],
                    AO.mult, AO.mult,
                )
                if scan:
                    for kb in range(2):
                        init = 0.0 if s0 == 0 else h8f[:, kb, s0 - 1:s0]
                        nc.vector.tensor_tensor_scan(
                            h8f[:, kb, s0:s0 + zs],
                            dd[:, kb, s0:s0 + zs],
                            gg[:, kb, s0:s0 + zs],
                            init, AO.mult, AO.add,
                        )

            nc.sync.dma_start(xT8[:, :, 0:512], xT8_d[:, :, 0:512])
            nc.sync.dma_start(w_in8l[:, 0], winl_d[:, 0])
            nc.sync.dma_start(w_in8[:, 1], win_d[:, 1])
            nc.sync.dma_start(w_in8l[:, 1], winl_d[:, 1])
            nc.sync.dma_start(w_f8[:], wf_d)
            nc.sync.dma_start(w_b8[:], wb_d)
            for si in range(len(SLABS)):
                if si + 2 in SLAB_DMA:
                    t0, ts = SLAB_DMA[si + 2]
                    nc.sync.dma_start(xT8[:, :, t0:t0 + ts],
                                      xT8_d[:, :, t0:t0 + ts])
                z_slab(si)
                f0, fs = FSEGS[si]
                dir_seg(f0, fs, f0, w_f8, d16, g16, scan=True)
                b0, bs = BSEGS[si]
                dir_seg(b0 + HALO, bs, b0, w_b8, d16b, g16b, scan=False)

            nc.sync.dma_start(w_o8[:], wo_d)
            for pc in (NCH2 - 1, NCH2 - 2):
                nc.sync.dma_start(r8p[:, pc], r8p_d[:, pc])

            # ---- bwd scans, descending (chains through seg inits) ----
            for sj in range(len(BSEGS) - 1, -1, -1):
                b0, bs = BSEGS[sj]
                hi = b0 + bs
                for kb in range(2):
                    init = 0.0 if hi == T_SCAN else h8b[:, kb, hi:hi + 1]
                    nc.vector.tensor_tensor_scan(
                        h8b[:, kb, b0:b0 + bs][:, ::-1],
                        d16b[:, kb, b0:b0 + bs][:, ::-1],
                        g16b[:, kb, b0:b0 + bs][:, ::-1],
                        init, AO.mult, AO.add,
                    )

            # ---- phase C: out chunks, 4 psum quarters each ----
            state = {}   # c -> (pqs, st, y16)
            order = list(range(NCH - 1, -1, -1))

            def gemm_q(pq, c, q, tok, csel, pc):
                dsl = slice(q * 512, (q + 1) * 512)
                nc.tensor.matmul(pq[:], h8f[:, :, tok], w_o8[:, 0:2, dsl],
                                 start=True, stop=False, perf_mode=DR)
                nc.tensor.matmul(pq[:], h8b[:, :, c * P:(c + 1) * P],
                                 w_o8[:, 2:4, dsl],
                                 start=False, stop=False, perf_mode=DR)
                # residual hi: DR transpose-extract of xT8 kblock pairs
                for j in range(4):
                    pb = q * 4 + (j // 2) * 2
                    sel = identhi8 if j % 2 == 0 else identlo8
                    nc.tensor.matmul(
                        pq[:, j * P:(j + 1) * P],
                        xT8[:, pb:pb + 2, tok],
                        sel[:],
                        start=False, stop=False,
                        perf_mode=DR, skip_group_check=True)
                # residual lo: DR select of the chunk pair plane
                nc.tensor.matmul(pq[:], csel[:], r8p[:, pc, :, dsl],
                                 start=False, stop=True,
                                 perf_mode=DR, skip_group_check=True)

            # st layout: 0 ss0, 1 ss1, 2 ss02, 3 ssum, 4 bias, 5 ss3,
            #            6 std, 7 rstd
            def stage0(c, prev):
                tok = slice(HALO + c * P, HALO + (c + 1) * P)
                csel = identhi8 if c % 2 == 0 else identlo8
                pc = c // 2
                st = stpool.tile([P, 8], f32, name="st")
                y16 = ypool.tile([P, 4, 512], f16, name="y16")
                pqs = []
                for q in range(4):
                    pq = psp.tile([P, 512], f32, name="pq", tag="ps")
                    pqs.append(pq)
                    gemm_q(pq, c, q, tok, csel, pc)
                    if q == 0:
                        nc.vector.tensor_tensor_reduce(
                            sqscrd[:], pq[:], pq[:], 1.0 / 1024.0,
                            0.0, AO.mult, AO.add, accum_out=st[:, 0:1],
                        )
                    elif q == 1:
                        nc.gpsimd.scalar_tensor_tensor(
                            sqscrp[:], pq[:], 1.0 / 1024.0, pq[:],
                            AO.mult, AO.mult, accum_out=st[:, 1:2],
                        )
                    elif q == 2:
                        nc.vector.tensor_tensor_reduce(
                            sqscrd[:], pq[:], pq[:], 1.0 / 1024.0,
                            st[:, 0:1], AO.mult, AO.add,
                            accum_out=st[:, 2:3],
                        )
                if prev is not None:
                    recip(prev)
                nc.vector.tensor_tensor(st[:, 3:4], st[:, 2:3], st[:, 1:2],
                                        AO.add)
                nc.vector.tensor_scalar(st[:, 4:5], st[:, 3:4], 0.5, EPS_S,
                                        AO.mult, op1=AO.add)
                state[c] = (pqs, st, y16)

            def stage2a(c):
                pqs, st, y16 = state[c]
                nc.scalar.activation(
                    sqscr[:], pqs[3][:], AF.Square, scale=1.0 / 32.0,
                    accum_out=st[:, 5:6],
                )
                nc.scalar.activation(st[:, 6:7], st[:, 5:6], AF.Sqrt,
                                     scale=0.5, bias=st[:, 4:5])

            def recip(c):
                st = state[c][1]
                nc.vector.reciprocal(st[:, 7:8], st[:, 6:7])

            def stage2b(c):
                pqs, st, y16 = state.pop(c)
                rstd = st[:, 7:8]
                nc.scalar.activation(y16[:, 0, :], pqs[0][:], AF.Copy,
                                     scale=rstd)
                nc.vector.tensor_scalar(y16[:, 1, :], pqs[1][:], rstd, None,
                                        AO.mult)
                nc.scalar.activation(y16[:, 2, :], pqs[2][:], AF.Copy,
                                     scale=rstd)
                nc.gpsimd.tensor_scalar(y16[:, 3, :], pqs[3][:], rstd, None,
                                        AO.mult)
                nc.sync.dma_start(y_d[c * P:(c + 1) * P, :], y16[:])

            for i in range(NCH + 2):
                if 1 <= i < NCH + 1:
                    stage2a(order[i - 1])
                if 2 <= i < NCH + 2:
                    stage2b(order[i - 2])
                if i < NCH:
                    c = order[i]
                    if c % 2 == 0 and c // 2 - 2 >= 0:
                        nc.sync.dma_start(r8p[:, c // 2 - 2],
                                          r8p_d[:, c // 2 - 2])
                    stage0(c, order[i - 1] if i >= 1 else None)
                elif i - 1 < NCH:
                    recip(order[i - 1])

    nc.compile()
    return nc


_NC_CACHE = {}


def _get_nc():
    if "nc" not in _NC_CACHE:
        _NC_CACHE["nc"] = build_nc()
    return _NC_CACHE["nc"]


def _prep_weights(W_in, W_fwd, W_bwd, W_out):
    W_in = np.asarray(W_in, np.float32)
    W_fwd = np.asarray(W_fwd, np.float32)
    W_bwd = np.asarray(W_bwd, np.float32)
    W_out = np.asarray(W_out, np.float32)
    # [P, 2, KD, 128]: w[p, m, j, i] = 16*W_in[j*128+p, m*128+i]
    w_in16s = (16.0 * W_in).reshape(KD, P, 2, DI // 2).transpose(1, 2, 0, 3)
    w_in16s = np.ascontiguousarray(w_in16s)
    w_in8 = w_in16s.astype(F8)
    w_in8l = (w_in16s - w_in8.astype(np.float32)).astype(
        ml_dtypes.float8_e5m2)
    w_f8 = (64.0 * W_fwd).reshape(2, P, 2 * DI).transpose(1, 0, 2)
    w_f8 = np.ascontiguousarray(w_f8).astype(F8)
    w_b8 = (64.0 * W_bwd).reshape(2, P, 2 * DI).transpose(1, 0, 2)
    w_b8 = np.ascontiguousarray(w_b8).astype(F8)
    # center each W_out row over its 2048 outputs -> out-term of y has
    # (analytically) zero mean; LN then needs no mean subtraction
    W_oc = W_out - W_out.mean(-1, keepdims=True)
    w_o8 = (16.0 * W_oc).reshape(4, P, D).transpose(1, 0, 2)
    w_o8 = np.ascontiguousarray(w_o8).astype(F8)
    return {
        "w_in8": w_in8, "w_in8l": np.ascontiguousarray(w_in8l),
        "w_f8": w_f8, "w_b8": w_b8, "w_o8": w_o8,
    }


def shard_inputs(x, W_in, W_fwd, W_bwd, W_out):
    xf = np.asarray(x, np.float32)
    xp = np.zeros((B, L + 2 * HALO, D), np.float32)
    xp[:, HALO:HALO + L] = xf
    wmaps = _prep_weights(W_in, W_fwd, W_bwd, W_out)
    in_maps = []
    for b in range(B):
        for h in range(2):
            ctx = xp[b, h * T_OWN:h * T_OWN + T_CTX]          # [T_CTX, D]
            xT8 = (32.0 * ctx.T).reshape(KD, P, T_CTX).transpose(1, 0, 2)
            xT8 = np.ascontiguousarray(xT8).astype(F8)        # [P, KD, T_CTX]
            own = xf[b, h * T_OWN:(h + 1) * T_OWN]            # [T_OWN, D]
            # residual: hi = x8h (from xT8); lo absorbs the per-token
            # centering: r = 32*(x - mean_d x) - x8h
            x8h = xT8.astype(np.float32).transpose(1, 0, 2).reshape(D, T_CTX)
            own8h = x8h[:, HALO:HALO + T_OWN].T               # [T_OWN, D] (32x)
            ownc = own - own.mean(-1, keepdims=True)
            r = 32.0 * ownc - own8h
            # pair-packed [P, NCH2, 2, D]: chunk 2pc+i, token p
            r8p = r.reshape(NCH2, 2, P, D).transpose(2, 0, 1, 3)
            r8p = np.ascontiguousarray(r8p).astype(F8)
            in_maps.append({"xT8": xT8, "r8p": r8p, **wmaps})
    return in_maps


def gather_outputs(results):
    out = np.empty((B, L, D), np.float32)
    for b in range(B):
        for h in range(2):
            out[b, h * T_OWN:(h + 1) * T_OWN] = (
                results[b * 2 + h]["y"].astype(np.float32)
            )
    return out


def run_on_hw(x, W_in, W_fwd, W_bwd, W_out, trace=False):
    from concourse.bass_utils import run_bass_kernel_spmd

    nc = _get_nc()
    in_maps = shard_inputs(x, W_in, W_fwd, W_bwd, W_out)
    res = run_bass_kernel_spmd(
        nc, in_maps, core_ids=list(range(N_CORES)), trace=trace
    )
    return gather_outputs(res.results), res


def kernel(x, W_in, W_fwd, W_bwd, W_out, gamma, beta):
    y, _ = run_on_hw(x, W_in, W_fwd, W_bwd, W_out)
    gamma = np.asarray(gamma, np.float32)
    beta = np.asarray(beta, np.float32)
    if not (np.all(gamma == 1.0) and np.all(beta == 0.0)):
        y = y * gamma + beta
    return y.astype(np.float32)


# revision 16
# speedup vs baseline: 1.2197x; 1.0728x over previous
"""Bidirectional simplified SSM kernel for Trainium2 (8 NeuronCores).

Math (per batch element b):
    z = x @ W_in                                  [L, DI]
    fwd:  o = z @ W_fwd; delta = sigmoid(o[:, :DI]); gate = o[:, DI:] * z
          h_t = delta_t * h_{t-1} + gate_t        (t ascending)
    bwd:  same with W_bwd, t descending
    y    = concat(h_fwd, h_bwd) @ W_out + x
    out  = LayerNorm(y) * gamma + beta

Sharding: 8 cores = 4 batches x 2 sequence halves with a 32-token halo on
each side (sigmoid gating decays ~0.5/step, so 32 warm-up steps reproduce
the cross-half scan state far below fp16 noise). No cross-core traffic.

v6 strategy (v1 = 99.4us):
- Zero-mean trick: host centers each W_out row over its 2048 outputs and
  folds per-token x-centering into the residual lo plane, so y has
  analytically zero mean and LayerNorm reduces to out = py * rstd (no
  mean pass, no center pass).
- All GEMMs fp8 DoubleRow (xT8 = fp8(32x) D-major; w_in8 + w_in8l e5m2
  residual = 16 W_in; Wf8/Wb8 = 64 W; scan state fp32 on DVE, h8 =
  fp8(8h); W_out8 = fp8(16 W'_out)). Residual adds are DoubleRow matmuls
  through stacked-identity selectors [4I;0]/[0;4I]: hi plane extracted
  from resident xT8 kblock pairs, lo plane from pair-packed r8p.
- Every PSUM tile is ONE bank (2KB) under a single 8-slot rotation: the
  Tile scheduler then never couples a slow consumer (Pool gate, norm) to
  an unrelated producer through slot reuse, and frees happen at the
  finest granularity.
- Phase A: 256-token slabs. Each slab does z GEMM + z8, the fwd
  o/sigmoid/gate/scan segment AND the bwd o/sigmoid/gate for the aligned
  bwd segment (scan-order-free), so all sigmoids finish before phase C's
  Sqrt cadence (one ACT table switch) and the bwd chain costs no
  dedicated phase. Only the serial bwd scans remain, overlapping phase
  C's first GEMMs.
- Phase C per chunk = 4 independent 512-col psum quarters. Squares:
  Q0/Q2 on DVE (tensor_tensor_reduce, init-chained), Q1 on Pool
  (scalar_tensor_tensor + accum), Q3 on ACT right when its GEMMs land;
  ACT Sqrt folds the other partials via bias = 0.5*(ss02+ss1) + eps
  (DVE smalls, off-chain); DVE recip; norms split ACT(Q0,Q2)/DVE(Q1)/
  Pool(Q3). Stats lag one pipeline stage, norms two, so each in-order
  engine queue only sees already-satisfied deps.
"""

import os
import sys

for _p in ("/opt/trn_rl_repo", "/root/.axon_site/_ro/trn_rl_repo"):
    if os.path.isdir(_p) and _p not in sys.path:
        sys.path.insert(0, _p)

import ml_dtypes
import numpy as np

import concourse.bacc as bacc
import concourse.mybir as mybir
import concourse.tile as tile
from concourse.masks import make_identity

P = 128
LN_EPS = 1e-5

B, L, D, DI = 4, 4096, 2048, 256
HALO = 32
T_OWN = L // 2            # tokens owned per core (2048)
T_CTX = T_OWN + 2 * HALO  # context incl. halos (2112)
T_SCAN = T_CTX - HALO     # tokens each direction scans over (2080)
N_CORES = 8

KD = D // P               # 16 k-blocks over D
NCH = T_OWN // P          # 16 output chunks
NCH2 = NCH // 2           # 8 chunk pairs for the lo-plane DR packing
SC = 128.0                # psum y scale (8*16)
EPS_S = SC * SC * LN_EPS

F8 = ml_dtypes.float8_e4m3

f8 = mybir.dt.float8e4
f8e5 = mybir.dt.float8e5
f16 = mybir.dt.float16
f32 = mybir.dt.float32
AO = mybir.AluOpType
AF = mybir.ActivationFunctionType
DR = mybir.MatmulPerfMode.DoubleRow

# 256-token z slabs (+64 halo tail); DMA in 512/576-token copies
SLABS = [(256 * k, 256) for k in range(8)] + [(2048, 64)]
SLAB_DMA = {0: (0, 512), 2: (512, 512), 4: (1024, 512), 6: (1536, 576)}
# fwd scan segs aligned to slabs (scan coords == z coords, 0..2080)
FSEGS = [(256 * k, 256) for k in range(8)] + [(2048, 32)]
# bwd segs in scan coords; z range = (b0+32, len), exactly slab si
BSEGS = [(0, 224)] + [(224 + 256 * m, 256) for m in range(7)] + [(2016, 64)]
# the same segs in reversed coords (rev[r] = fwd[T_SCAN-1-r]), ascending
RSEGS = sorted((T_SCAN - b0 - bs, bs) for b0, bs in BSEGS)


def build_nc():
    nc = bacc.Bacc("TRN2", target_bir_lowering=False, debug=False)
    xT8_d = nc.dram_tensor("xT8", [P, KD, T_CTX], f8, kind="ExternalInput").ap()
    r8p_d = nc.dram_tensor("r8p", [P, NCH2, 2, D], f8,
                           kind="ExternalInput").ap()
    win_d = nc.dram_tensor("w_in8", [P, 2, KD, DI // 2], f8,
                           kind="ExternalInput").ap()
    winl_d = nc.dram_tensor("w_in8l", [P, 2, KD, DI // 2], f8e5,
                            kind="ExternalInput").ap()
    wf_d = nc.dram_tensor("w_f8", [P, 2, 2 * DI], f8, kind="ExternalInput").ap()
    wb_d = nc.dram_tensor("w_b8", [P, 2, 2 * DI], f8, kind="ExternalInput").ap()
    wo_d = nc.dram_tensor("w_o8", [P, 4, D], f8, kind="ExternalInput").ap()
    y_d = nc.dram_tensor("y", [T_OWN, D], f16, kind="ExternalOutput").ap()

    with tile.TileContext(nc) as tc:
        with (
            tc.tile_pool(name="const", bufs=1) as cpool,
            tc.tile_pool(name="big", bufs=1) as bpool,
            tc.tile_pool(name="ych", bufs=3) as ypool,
            tc.tile_pool(name="st", bufs=4) as stpool,
            tc.tile_pool(name="ps", bufs=8, space="PSUM") as psp,
        ):
            # ---- constants / weights ----
            w_in8 = cpool.tile([P, 2, KD, DI // 2], f8)
            w_in8l = cpool.tile([P, 2, KD, DI // 2], f8e5)
            w_f8 = cpool.tile([P, 2, 2 * DI], f8)
            w_b8 = cpool.tile([P, 2, 2 * DI], f8)
            w_o8 = cpool.tile([P, 4, D], f8)
            ident = cpool.tile([P, P], f16)
            identhi8 = cpool.tile([P, 2, P], f8)   # [4I; 0]
            identlo8 = cpool.tile([P, 2, P], f8)   # [0; 4I]
            nc.sync.dma_start(w_in8[:, 0], win_d[:, 0])
            make_identity(nc, ident[:])
            nc.vector.memset(identhi8[:], 0.0)
            nc.vector.memset(identlo8[:], 0.0)
            nc.vector.tensor_scalar(identhi8[:, 0], ident[:], 4.0, None,
                                    AO.mult)
            nc.vector.tensor_scalar(identlo8[:, 1], ident[:], 4.0, None,
                                    AO.mult)

            # ---- big SBUF state ----
            xT8 = bpool.tile([P, KD, T_CTX], f8)
            r8p = bpool.tile([P, NCH2, 2, D], f8)
            z8 = bpool.tile([P, 2, T_CTX], f8)
            d16 = bpool.tile([P, 2, T_SCAN], f16)   # fwd delta
            g16 = bpool.tile([P, 2, T_SCAN], f16)   # fwd gate
            # bwd arrays in REVERSED token order (rev[r] = fwd[T_SCAN-1-r]):
            # the reversal happens once in the bwd o-GEMM's moving AP (free),
            # so sigmoid/gate/scan all run on contiguous forward APs.
            d16b = bpool.tile([P, 2, T_SCAN], f16)  # bwd delta (rev)
            g16b = bpool.tile([P, 2, T_SCAN], f16)  # bwd gate (rev)
            h8f = bpool.tile([P, 2, T_SCAN], f8)
            h8b = bpool.tile([P, 2, T_SCAN], f8)    # bwd scan state (rev)
            sqscr = bpool.tile([P, 512], f16)       # ACT square scratch
            sqscrd = bpool.tile([P, 512], f16)      # DVE square scratch
            sqscrp = bpool.tile([P, 512], f16)      # Pool square scratch

            # ---- phase A: z GEMM + fwd segs + bwd part1, slab by slab ----
            def z_slab(si):
                t0, ts = SLABS[si]
                pz = psp.tile([P, 2, 256], f32, name="pz", tag="ps")
                for m in range(2):
                    for j in range(KD // 2):
                        nc.tensor.matmul(
                            pz[:, m, :ts],
                            w_in8[:, m, 2 * j:2 * j + 2, :],
                            xT8[:, 2 * j:2 * j + 2, t0:t0 + ts],
                            start=(j == 0), stop=False,
                            perf_mode=DR,
                        )
                    for j in range(KD // 2):
                        nc.tensor.matmul(
                            pz[:, m, :ts],
                            w_in8l[:, m, 2 * j:2 * j + 2, :],
                            xT8[:, 2 * j:2 * j + 2, t0:t0 + ts],
                            start=False, stop=(j == KD // 2 - 1),
                            perf_mode=DR, skip_group_check=True,
                        )
                nc.scalar.activation(z8[:, :, t0:t0 + ts], pz[:, :, :ts],
                                     AF.Copy, scale=1.0 / 32.0)

            def dir_seg(z0, zs, s0, wdir, dd, gg, rev):
                """o GEMM + sigmoid + gate for z8[z0:z0+zs] -> coords s0..

                rev=True reverses the token order via the o-GEMM moving AP
                (and the gate's z operand) so downstream ops stay forward.
                """
                zin = z8[:, :, z0:z0 + zs]
                if rev:
                    zin = zin[:, :, ::-1]
                pod = psp.tile([P, 2, 256], f32, name="pod", tag="ps")
                pog = psp.tile([P, 2, 256], f32, name="pog", tag="ps")
                for m2 in range(4):
                    dst = pod if m2 < 2 else pog
                    nc.tensor.matmul(
                        dst[:, m2 % 2, :zs],
                        wdir[:, :, m2 * P:(m2 + 1) * P],
                        zin,
                        start=True, stop=True, perf_mode=DR,
                    )
                nc.scalar.activation(
                    dd[:, :, s0:s0 + zs], pod[:, :, :zs], AF.Sigmoid,
                    scale=1.0 / 1024.0,
                )
                nc.gpsimd.scalar_tensor_tensor(
                    gg[:, :, s0:s0 + zs], pog[:, :, :zs],
                    1.0 / 2048.0, zin,
                    AO.mult, AO.mult,
                )

            def fwd_scan(s0, ss):
                for kb in range(2):
                    init = 0.0 if s0 == 0 else h8f[:, kb, s0 - 1:s0]
                    nc.vector.tensor_tensor_scan(
                        h8f[:, kb, s0:s0 + ss],
                        d16[:, kb, s0:s0 + ss],
                        g16[:, kb, s0:s0 + ss],
                        init, AO.mult, AO.add,
                    )

            def emit_segs(si):
                f0, fs = FSEGS[si]
                dir_seg(f0, fs, f0, w_f8, d16, g16, rev=False)
                fwd_scan(f0, fs)
                b0, bs = BSEGS[si]
                dir_seg(b0 + HALO, bs, T_SCAN - b0 - bs, w_b8, d16b, g16b,
                        rev=True)

            nc.sync.dma_start(xT8[:, :, 0:512], xT8_d[:, :, 0:512])
            nc.sync.dma_start(w_in8l[:, 0], winl_d[:, 0])
            nc.sync.dma_start(w_in8[:, 1], win_d[:, 1])
            nc.sync.dma_start(w_in8l[:, 1], winl_d[:, 1])
            nc.sync.dma_start(w_f8[:], wf_d)
            nc.sync.dma_start(w_b8[:], wb_d)
            for si in range(len(SLABS)):
                if si + 2 in SLAB_DMA:
                    t0, ts = SLAB_DMA[si + 2]
                    nc.sync.dma_start(xT8[:, :, t0:t0 + ts],
                                      xT8_d[:, :, t0:t0 + ts])
                z_slab(si)
                if si > 0:
                    emit_segs(si - 1)
            emit_segs(len(SLABS) - 1)

            nc.sync.dma_start(w_o8[:], wo_d)
            for pc in (NCH2 - 1, NCH2 - 2):
                nc.sync.dma_start(r8p[:, pc], r8p_d[:, pc])

            # ---- bwd scans, ascending in reversed coords ----
            for r0, rs in RSEGS:
                for kb in range(2):
                    init = 0.0 if r0 == 0 else h8b[:, kb, r0 - 1:r0]
                    nc.vector.tensor_tensor_scan(
                        h8b[:, kb, r0:r0 + rs],
                        d16b[:, kb, r0:r0 + rs],
                        g16b[:, kb, r0:r0 + rs],
                        init, AO.mult, AO.add,
                    )

            # ---- phase C: out chunks, 4 psum quarters each ----
            state = {}   # c -> (pqs, st, y16)
            order = list(range(NCH - 1, -1, -1))

            def gemm_q(pq, c, q, tok, csel, pc):
                dsl = slice(q * 512, (q + 1) * 512)
                nc.tensor.matmul(pq[:], h8f[:, :, tok], w_o8[:, 0:2, dsl],
                                 start=True, stop=False, perf_mode=DR)
                hb = h8b[:, :, T_SCAN - (c + 1) * P:T_SCAN - c * P]
                nc.tensor.matmul(pq[:], hb[:, :, ::-1], w_o8[:, 2:4, dsl],
                                 start=False, stop=False, perf_mode=DR)
                # residual hi: DR transpose-extract of xT8 kblock pairs
                for j in range(4):
                    pb = q * 4 + (j // 2) * 2
                    sel = identhi8 if j % 2 == 0 else identlo8
                    nc.tensor.matmul(
                        pq[:, j * P:(j + 1) * P],
                        xT8[:, pb:pb + 2, tok],
                        sel[:],
                        start=False, stop=False,
                        perf_mode=DR, skip_group_check=True)
                # residual lo: DR select of the chunk pair plane
                nc.tensor.matmul(pq[:], csel[:], r8p[:, pc, :, dsl],
                                 start=False, stop=True,
                                 perf_mode=DR, skip_group_check=True)

            # st layout: 0 ss0, 1 ss1, 2 ss02, 3 ssum, 4 bias, 5 ss3,
            #            6 std, 7 rstd
            def stage0(c, prev):
                tok = slice(HALO + c * P, HALO + (c + 1) * P)
                csel = identhi8 if c % 2 == 0 else identlo8
                pc = c // 2
                st = stpool.tile([P, 8], f32, name="st")
                y16 = ypool.tile([P, 4, 512], f16, name="y16")
                pqs = []
                for q in range(4):
                    pq = psp.tile([P, 512], f32, name="pq", tag="ps")
                    pqs.append(pq)
                    gemm_q(pq, c, q, tok, csel, pc)
                    if q == 0:
                        nc.vector.tensor_tensor_reduce(
                            sqscrd[:], pq[:], pq[:], 1.0 / 1024.0,
                            0.0, AO.mult, AO.add, accum_out=st[:, 0:1],
                        )
                    elif q == 1:
                        nc.gpsimd.scalar_tensor_tensor(
                            sqscrp[:], pq[:], 1.0 / 1024.0, pq[:],
                            AO.mult, AO.mult, accum_out=st[:, 1:2],
                        )
                    elif q == 2:
                        nc.vector.tensor_tensor_reduce(
                            sqscrd[:], pq[:], pq[:], 1.0 / 1024.0,
                            st[:, 0:1], AO.mult, AO.add,
                            accum_out=st[:, 2:3],
                        )
                if prev is not None:
                    recip(prev)
                nc.vector.tensor_tensor(st[:, 3:4], st[:, 2:3], st[:, 1:2],
                                        AO.add)
                nc.vector.tensor_scalar(st[:, 4:5], st[:, 3:4], 0.5, EPS_S,
                                        AO.mult, op1=AO.add)
                state[c] = (pqs, st, y16)

            def stage2a(c):
                pqs, st, y16 = state[c]
                nc.scalar.activation(
                    sqscr[:], pqs[3][:], AF.Square, scale=1.0 / 32.0,
                    accum_out=st[:, 5:6],
                )
                nc.scalar.activation(st[:, 6:7], st[:, 5:6], AF.Sqrt,
                                     scale=0.5, bias=st[:, 4:5])

            def recip(c):
                st = state[c][1]
                nc.vector.reciprocal(st[:, 7:8], st[:, 6:7])

            def stage2b(c):
                pqs, st, y16 = state.pop(c)
                rstd = st[:, 7:8]
                nc.scalar.activation(y16[:, 0, :], pqs[0][:], AF.Copy,
                                     scale=rstd)
                nc.vector.tensor_scalar(y16[:, 1, :], pqs[1][:], rstd, None,
                                        AO.mult)
                nc.scalar.activation(y16[:, 2, :], pqs[2][:], AF.Copy,
                                     scale=rstd)
                nc.gpsimd.tensor_scalar(y16[:, 3, :], pqs[3][:], rstd, None,
                                        AO.mult)
                nc.sync.dma_start(y_d[c * P:(c + 1) * P, :], y16[:])

            for i in range(NCH + 2):
                if 1 <= i < NCH + 1:
                    stage2a(order[i - 1])
                if 2 <= i < NCH + 2:
                    stage2b(order[i - 2])
                if i < NCH:
                    c = order[i]
                    if c % 2 == 0 and c // 2 - 2 >= 0:
                        nc.sync.dma_start(r8p[:, c // 2 - 2],
                                          r8p_d[:, c // 2 - 2])
                    stage0(c, order[i - 1] if i >= 1 else None)
                elif i - 1 < NCH:
                    recip(order[i - 1])

    nc.compile()
    return nc


_NC_CACHE = {}


def _get_nc():
    if "nc" not in _NC_CACHE:
        _NC_CACHE["nc"] = build_nc()
    return _NC_CACHE["nc"]


def _prep_weights(W_in, W_fwd, W_bwd, W_out):
    W_in = np.asarray(W_in, np.float32)
    W_fwd = np.asarray(W_fwd, np.float32)
    W_bwd = np.asarray(W_bwd, np.float32)
    W_out = np.asarray(W_out, np.float32)
    # [P, 2, KD, 128]: w[p, m, j, i] = 16*W_in[j*128+p, m*128+i]
    w_in16s = (16.0 * W_in).reshape(KD, P, 2, DI // 2).transpose(1, 2, 0, 3)
    w_in16s = np.ascontiguousarray(w_in16s)
    w_in8 = w_in16s.astype(F8)
    w_in8l = (w_in16s - w_in8.astype(np.float32)).astype(
        ml_dtypes.float8_e5m2)
    w_f8 = (64.0 * W_fwd).reshape(2, P, 2 * DI).transpose(1, 0, 2)
    w_f8 = np.ascontiguousarray(w_f8).astype(F8)
    w_b8 = (64.0 * W_bwd).reshape(2, P, 2 * DI).transpose(1, 0, 2)
    w_b8 = np.ascontiguousarray(w_b8).astype(F8)
    # center each W_out row over its 2048 outputs -> out-term of y has
    # (analytically) zero mean; LN then needs no mean subtraction
    W_oc = W_out - W_out.mean(-1, keepdims=True)
    w_o8 = (16.0 * W_oc).reshape(4, P, D).transpose(1, 0, 2)
    w_o8 = np.ascontiguousarray(w_o8).astype(F8)
    return {
        "w_in8": w_in8, "w_in8l": np.ascontiguousarray(w_in8l),
        "w_f8": w_f8, "w_b8": w_b8, "w_o8": w_o8,
    }


def shard_inputs(x, W_in, W_fwd, W_bwd, W_out):
    xf = np.asarray(x, np.float32)
    xp = np.zeros((B, L + 2 * HALO, D), np.float32)
    xp[:, HALO:HALO + L] = xf
    wmaps = _prep_weights(W_in, W_fwd, W_bwd, W_out)
    in_maps = []
    for b in range(B):
        for h in range(2):
            ctx = xp[b, h * T_OWN:h * T_OWN + T_CTX]          # [T_CTX, D]
            xT8 = (32.0 * ctx.T).reshape(KD, P, T_CTX).transpose(1, 0, 2)
            xT8 = np.ascontiguousarray(xT8).astype(F8)        # [P, KD, T_CTX]
            own = xf[b, h * T_OWN:(h + 1) * T_OWN]            # [T_OWN, D]
            # residual: hi = x8h (from xT8); lo absorbs the per-token
            # centering: r = 32*(x - mean_d x) - x8h
            x8h = xT8.astype(np.float32).transpose(1, 0, 2).reshape(D, T_CTX)
            own8h = x8h[:, HALO:HALO + T_OWN].T               # [T_OWN, D] (32x)
            ownc = own - own.mean(-1, keepdims=True)
            r = 32.0 * ownc - own8h
            # pair-packed [P, NCH2, 2, D]: chunk 2pc+i, token p
            r8p = r.reshape(NCH2, 2, P, D).transpose(2, 0, 1, 3)
            r8p = np.ascontiguousarray(r8p).astype(F8)
            in_maps.append({"xT8": xT8, "r8p": r8p, **wmaps})
    return in_maps


def gather_outputs(results):
    out = np.empty((B, L, D), np.float32)
    for b in range(B):
        for h in range(2):
            out[b, h * T_OWN:(h + 1) * T_OWN] = (
                results[b * 2 + h]["y"].astype(np.float32)
            )
    return out


def run_on_hw(x, W_in, W_fwd, W_bwd, W_out, trace=False):
    from concourse.bass_utils import run_bass_kernel_spmd

    nc = _get_nc()
    in_maps = shard_inputs(x, W_in, W_fwd, W_bwd, W_out)
    res = run_bass_kernel_spmd(
        nc, in_maps, core_ids=list(range(N_CORES)), trace=trace
    )
    return gather_outputs(res.results), res


def kernel(x, W_in, W_fwd, W_bwd, W_out, gamma, beta):
    y, _ = run_on_hw(x, W_in, W_fwd, W_bwd, W_out)
    gamma = np.asarray(gamma, np.float32)
    beta = np.asarray(beta, np.float32)
    if not (np.all(gamma == 1.0) and np.all(beta == 0.0)):
        y = y * gamma + beta
    return y.astype(np.float32)
